# revision 32
# baseline (speedup 1.0000x reference)
"""Trainium2 Bass kernel for nn_CausalSelfAttention_8443905704568.

Causal self-attention with RoPE + 10-token adapter cross-attention,
B=1, T=2048, C=4096, H=32 heads of hd=128, fp32 I/O.

Strategy: tensor-parallel over heads across 8 NeuronCores (4 heads/core).
Each core computes qkv for its heads (w_attn rows sharded), runs
flash-style attention in transposed orientation (scores^T, so the
probs land partition=key which feeds the P^T @ V matmul directly),
the adapter cross-attention, and a partial output projection
(w_proj columns sharded). Host sums the 8 partial outputs.

v2 layout: everything bf16 (PE does 1 cyc/col at any moving-dim size,
half the HBM traffic of f32r; end-to-end error ~1e-3 vs the 2e-2 gate).
One fused pass over x computes q, k and v per t-block with all three
weight matrices SBUF-resident; q/k (roped) and v stay in SBUF for the
attention phase -- no DRAM staging round-trip at all.  All device
inputs are host-prepacked into [128, ...] partition-major tiles so
every DMA is a full-line contiguous copy.  w_proj streams in during
attention; the projection writes bf16 partials that the host sums.

Perf structure:
- startup: x block 0 chunks interleave with the first wqk chunks at the
  head of the DMA queue; the exp activation table is warmed by a dummy
  exp during that wait; the tiny adapter-column strip DMAs (2048 20B
  descriptors) are pushed behind the bulk weight transfers
- fused pass: per t-block q heads -> k heads -> v sub-tiles; tb0 runs
  ct-outer on q so matmuls start after the first weight chunk; adapter
  K rides tb0's k matmuls as 10 extra moving columns; 4 x-block
  buffers so the next block's DMA runs under the current compute
- attention: diagonal j-tiles are processed first and restricted to
  their valid i-columns (only a [128,128] triangle mask remains, on
  the vector engine); scores pairs are software-pipelined two ahead of
  their exp; the denominator rides one ones-matmul per pair on a
  DVE-merged eT sum; softmax normalization (reciprocal via a [1,1024]
  -> [128,8] DMA reshape, gating folded in per-partition) is deferred
  one unit so the PE never waits on it; w_proj streams in 0.5MB chunks
  between the first attention units
- projection: 4-deep PSUM rotation, bf16 partial out flushed per half
  row-tile

Everything is hardcoded for the shapes above; host-side prep only
slices/transposes/casts inputs (layout, not model compute).
"""

import math
import os

import ml_dtypes
import numpy as np

import concourse.bass as bass
import concourse.mybir as mybir
import concourse.tile as tile
from concourse import bacc
from concourse.bass_utils import run_bass_kernel_spmd

F32 = mybir.dt.float32
BF16 = mybir.dt.bfloat16

T = 2048
C = 4096
NHEAD_TOTAL = 32
NCORES = 8
NH = NHEAD_TOTAL // NCORES      # heads per core = 4
HD = C // NHEAD_TOTAL           # head dim = 128
P = 128
CT = C // P                     # contraction tiles = 32
OW = NH * HD                    # per-pass weight output dim = 512
TB = 256                        # x t-block
NTB = T // TB                   # 8
IB = 512                        # attention i-block
NIB = T // IB                   # 4
NJT = T // P                    # 16 j-tiles
AT = 10                         # adapter tokens
SCALE = 1.0 / math.sqrt(HD)
ASCALE = 1.0 / math.sqrt(C)
EXP = mybir.ActivationFunctionType.Exp
CQ = CT // 4                    # weight DMA chunk = 8 ct tiles


def _build_nc(dbg=False):
    nc = bacc.Bacc("TRN2", target_bir_lowering=False, debug=False,
                   num_devices=NCORES)

    xblk_d = nc.dram_tensor("xblk", [P, NTB, CT, TB], BF16,
                            kind="ExternalInput").ap()
    wqkT_d = nc.dram_tensor("wqkT", [P, CT, 2 * OW], BF16,
                            kind="ExternalInput").ap()
    wvT_d = nc.dram_tensor("wvT", [P, CT, OW], BF16,
                           kind="ExternalInput").ap()
    wpT_d = nc.dram_tensor("wpT", [P, NH, C], BF16,
                           kind="ExternalInput").ap()
    awteT_d = nc.dram_tensor("awteT", [P, CT, AT], BF16,
                             kind="ExternalInput").ap()
    cosT_d = nc.dram_tensor("cosT", [P, T], F32, kind="ExternalInput").ap()
    sinT_d = nc.dram_tensor("sinT", [P, T], F32, kind="ExternalInput").ap()
    rotT_d = nc.dram_tensor("rotT", [P, P], BF16, kind="ExternalInput").ap()
    tri_d = nc.dram_tensor("tri", [P, P], BF16, kind="ExternalInput").ap()
    mask3_d = nc.dram_tensor("mask3", [P, IB], BF16,
                             kind="ExternalInput").ap()
    gcol_d = nc.dram_tensor("gcol", [P, 1], F32, kind="ExternalInput").ap()

    out_d = nc.dram_tensor("out", [T, C], BF16, kind="ExternalOutput").ap()
    if dbg:
        qdbg_d = nc.dram_tensor("qdbg", [P, NH, T], BF16,
                                kind="ExternalOutput").ap()
        kdbg_d = nc.dram_tensor("kdbg", [P, NH, T], BF16,
                                kind="ExternalOutput").ap()
        vdbg_d = nc.dram_tensor("vdbg", [P, NJT, OW], BF16,
                                kind="ExternalOutput").ap()
        akdbg_d = nc.dram_tensor("akdbg", [P, NH, AT], BF16,
                                 kind="ExternalOutput").ap()
        avdbg_d = nc.dram_tensor("avdbg", [AT, OW], BF16,
                                 kind="ExternalOutput").ap()
        ydbg_d = nc.dram_tensor("ydbg", [P, NH, T], BF16,
                                kind="ExternalOutput").ap()

    with tile.TileContext(nc) as tc:
      # persistent staging: q/k (roped) + v + y, all bf16, SBUF-resident
      with tc.tile_pool(name="stage", bufs=1) as stage, \
           tc.tile_pool(name="const", bufs=1) as cpool:
        qT_sb = stage.tile([P, NH, T], BF16, name="qT_sb")
        kT_sb = stage.tile([P, NH, T], BF16, name="kT_sb")
        v_sb = stage.tile([P, NJT, OW], BF16, name="v_sb")
        yT_sb = stage.tile([P, NH, T], BF16, name="yT_sb")

        awteT_sb = cpool.tile([P, CT, AT], BF16, name="awteT_sb")
        gcol_sb = cpool.tile([P, 1], F32, name="gcol_sb")
        ones128f = cpool.tile([P, 1], F32, name="ones128f")
        ones1f = cpool.tile([1, P], F32, name="ones1f")
        ones128 = cpool.tile([P, 1], BF16, name="ones128")
        ones1 = cpool.tile([1, P], BF16, name="ones1")
        akT_sb = cpool.tile([P, NH, AT], BF16, name="akT_sb")
        av_sb = cpool.tile([AT, OW], BF16, name="av_sb")
        tri_sb = cpool.tile([P, P], BF16, name="tri_sb")
        mask3_sb = cpool.tile([P, IB], BF16, name="mask3_sb")
        warm_sb = cpool.tile([1, 8], BF16, name="warm_sb")

        # ===== fused qkv pass (x read once, weights all resident) ========
        with tc.tile_pool(name="arot", bufs=1) as arot_pool, \
             tc.tile_pool(name="wpool", bufs=1) as w_pool, \
             tc.tile_pool(name="cs", bufs=2) as cs_pool, \
             tc.tile_pool(name="xa", bufs=4) as xa_pool, \
             tc.tile_pool(name="ropeA", bufs=2) as rope_pool, \
             tc.tile_pool(name="psA", bufs=4, space="PSUM") as psA, \
             tc.tile_pool(name="psArot", bufs=2, space="PSUM") as psArot, \
             tc.tile_pool(name="psAv", bufs=2, space="PSUM") as psAv:

            wqk_sb = w_pool.tile([P, CT, 2 * OW], BF16, name="wqk_sb")
            wv_sb = w_pool.tile([P, CT, OW], BF16, name="wv_sb")
            rotT_sb = arot_pool.tile([P, P], BF16, name="rotT_sb")

            def load_cs(tsl):
                cos_t = cs_pool.tile([P, TB], F32, tag="cos", name="cos_t")
                sin_t = cs_pool.tile([P, TB], F32, tag="sin", name="sin_t")
                nc.sync.dma_start(cos_t[:], cosT_d[:, tsl])
                nc.sync.dma_start(sin_t[:], sinT_d[:, tsl])
                return cos_t, sin_t

            def load_x_block(tb, chunks=1):
                # tb0 tiles carry the 10 adapter-wte columns appended to the
                # x columns, so adapter K rides the k matmuls as extra
                # moving columns of the same accumulation group
                wide = (tb == 0)
                ncol = TB + AT if wide else TB
                xh = []
                for half in range(2):
                    base = half * (CT // 2)
                    xa = xa_pool.tile([P, CT // 2, ncol], BF16, tag="xa",
                                      name="xa")
                    cn = (CT // 2) // chunks
                    for c in range(chunks):
                        nc.sync.dma_start(
                            xa[:, c * cn:(c + 1) * cn, :TB],
                            xblk_d[:, tb, base + c * cn:base + (c + 1) * cn,
                                   :])
                    xh.append(xa)
                return xh

            def load_adapter_strips(xh):
                # 10-wide strips -> tiny DMA descriptors; issued after the
                # weight chunks so they never sit ahead of them in the queue
                for half in range(2):
                    base = half * (CT // 2)
                    nc.sync.dma_start(xh[half][:, :, TB:],
                                      awteT_d[:, base:base + CT // 2, :])

            # DMA issue order: x block 0 + first wqk chunks fine-grained
            # (what the first matmuls wait on), then everything else.
            xh0 = []
            for half in range(2):
                xa = xa_pool.tile([P, CT // 2, TB + AT], BF16, tag="xa",
                                  name="xa")
                xh0.append(xa)
            for c in range(4):
                nc.sync.dma_start(
                    xh0[0][:, c * 4:(c + 1) * 4, :TB],
                    xblk_d[:, 0, c * 4:(c + 1) * 4, :])
                nc.sync.dma_start(wqk_sb[:, bass.ts(c, 2), :],
                                  wqkT_d[:, bass.ts(c, 2), :])
                nc.sync.dma_start(
                    xh0[1][:, c * 4:(c + 1) * 4, :TB],
                    xblk_d[:, 0, 16 + c * 4:16 + (c + 1) * 4, :])
            cs0 = load_cs(bass.ts(0, TB))
            for c4 in range(1, 4):
                nc.sync.dma_start(wqk_sb[:, bass.ts(c4, CQ), :],
                                  wqkT_d[:, bass.ts(c4, CQ), :])
            for c4 in range(4):
                nc.sync.dma_start(wv_sb[:, bass.ts(c4, CQ), :],
                                  wvT_d[:, bass.ts(c4, CQ), :])
            nc.sync.dma_start(rotT_sb[:], rotT_d[:])
            load_adapter_strips(xh0)
            nc.sync.dma_start(awteT_sb[:], awteT_d[:])
            nc.sync.dma_start(tri_sb[:], tri_d[:])
            nc.sync.dma_start(mask3_sb[:], mask3_d[:])
            nc.sync.dma_start(gcol_sb[:], gcol_d[:])

            nc.vector.memset(ones128f[:], 1.0)
            nc.vector.memset(ones1f[:], 1.0)
            nc.vector.tensor_copy(out=ones128[:], in_=ones128f[:])
            nc.vector.tensor_copy(out=ones1[:], in_=ones1f[:])
            # warm the exp activation-table set during the initial DMA wait
            nc.scalar.activation(warm_sb[:], ones1f[:, :8], EXP)

            def rope_tail(pqk, dst, oh, tsl, wide, cos_t, sin_t):
                # dst[:, oh, tsl] = pqk[:, :TB]*cos + rot(pqk)*sin
                if wide:
                    nc.scalar.copy(akT_sb[:, oh, :], pqk[:, TB:])
                raw = rope_pool.tile([P, TB], BF16, tag="raw", name="raw")
                nc.scalar.copy(raw[:], pqk[:, :TB])
                prot = psArot.tile([P, TB], F32, tag="prot", name="prot")
                nc.tensor.matmul(prot[:], rotT_sb[:], raw[:],
                                 start=True, stop=True)
                t1 = rope_pool.tile([P, TB], BF16, tag="t1", name="t1")
                nc.vector.tensor_mul(t1[:], prot[:], sin_t[:])
                dsl = dst[:, oh, tsl]
                nc.vector.tensor_mul(dsl, pqk[:, :TB], cos_t[:])
                nc.vector.tensor_add(dsl, dsl, t1[:])

            for tb in range(NTB):
                tsl = bass.ts(tb, TB)
                xh = xh0 if tb == 0 else load_x_block(tb)
                wide = (tb == 0)
                cos_t, sin_t = cs0 if tb == 0 else load_cs(tsl)

                # ---- q heads ----
                if tb == 0:
                    # ct-outer: matmuls start as soon as the first weight
                    # chunk + x half arrive
                    pqks = [psA.tile([P, TB], F32, tag="pqk",
                                     name=f"pqk{oh}") for oh in range(NH)]
                    for ct in range(CT):
                        for oh in range(NH):
                            nc.tensor.matmul(
                                pqks[oh][:],
                                wqk_sb[:, ct, oh * HD:(oh + 1) * HD],
                                xh[ct // (CT // 2)][:, ct % (CT // 2), :TB],
                                start=(ct == 0), stop=(ct == CT - 1))
                    for oh in range(NH):
                        rope_tail(pqks[oh], qT_sb, oh, tsl, False, cos_t, sin_t)
                else:
                    for oh in range(NH):
                        pqk = psA.tile([P, TB], F32, tag="pqk", name="pqk")
                        for ct in range(CT):
                            nc.tensor.matmul(
                                pqk[:],
                                wqk_sb[:, ct, oh * HD:(oh + 1) * HD],
                                xh[ct // (CT // 2)][:, ct % (CT // 2), :TB],
                                start=(ct == 0), stop=(ct == CT - 1))
                        rope_tail(pqk, qT_sb, oh, tsl, False, cos_t, sin_t)

                # ---- k heads (adapter K rides tb0 as 10 extra columns) ----
                ncol = TB + AT if wide else TB
                for oh in range(NH):
                    pqk = psA.tile([P, ncol], F32, tag="pqk", name="pqkk")
                    for ct in range(CT):
                        nc.tensor.matmul(
                            pqk[:],
                            wqk_sb[:, ct, OW + oh * HD:OW + (oh + 1) * HD],
                            xh[ct // (CT // 2)][:, ct % (CT // 2), :],
                            start=(ct == 0), stop=(ct == CT - 1))
                    rope_tail(pqk, kT_sb, oh, tsl, wide, cos_t, sin_t)

                # ---- v sub-tiles ----
                for sub in range(TB // P):
                    tt = tb * (TB // P) + sub
                    pv = psAv.tile([P, OW], F32, tag="pv", name="pv")
                    for ct in range(CT):
                        nc.tensor.matmul(
                            pv[:],
                            xh[ct // (CT // 2)][:, ct % (CT // 2),
                                                sub * P:(sub + 1) * P],
                            wv_sb[:, ct, :],
                            start=(ct == 0), stop=(ct == CT - 1))
                    nc.scalar.copy(v_sb[:, tt, :], pv[:])

            # adapter-v
            pav = psAv.tile([AT, OW], F32, tag="pv", name="pav")
            for ct in range(CT):
                nc.tensor.matmul(pav[:], awteT_sb[:, ct, :], wv_sb[:, ct, :],
                                 start=(ct == 0), stop=(ct == CT - 1))
            nc.scalar.copy(av_sb[:], pav[:])

        if dbg:
            nc.sync.dma_start(qdbg_d[:], qT_sb[:])
            nc.sync.dma_start(kdbg_d[:], kT_sb[:])
            nc.sync.dma_start(vdbg_d[:], v_sb[:])
            nc.sync.dma_start(akdbg_d[:], akT_sb[:])
            nc.sync.dma_start(avdbg_d[:], av_sb[:])

        # ================= attention per head (all SBUF-resident) =========
        with tc.tile_pool(name="wp2", bufs=1) as wp2_pool:
          wp_sb = wp2_pool.tile([P, NH, C], BF16, name="wp_sb")

          def load_wp_chunk(i):
              # 0.5MB chunks interleaved between attention units so they
              # never delay the small normalization DMAs behind them
              hh, half = divmod(i, 2)
              csl = bass.ts(half, C // 2)
              nc.sync.dma_start(wp_sb[:, hh, csl], wpT_d[:, hh, csl])

          with tc.tile_pool(name="expp", bufs=6) as exp_pool, \
               tc.tile_pool(name="small", bufs=2) as small_pool, \
               tc.tile_pool(name="bcast", bufs=2) as bc_pool, \
               tc.tile_pool(name="psST", bufs=2, space="PSUM") as psST, \
               tc.tile_pool(name="psYT", bufs=2, space="PSUM") as psYT, \
               tc.tile_pool(name="psDen", bufs=1, space="PSUM") as psDen, \
               tc.tile_pool(name="psYa", bufs=1, space="PSUM") as psYa:

              pending = [None]  # deferred normalization of the previous block

              def emit_norm_stats(pyT, pyaT, pden, paden, hh, ibb):
                  # off the PE critical path: both denominators go into one
                  # [1, 2*512] row, reshaped to [128, 8] by DMA so the
                  # reciprocal runs on all lanes (self-denominators land on
                  # partitions 0-63, adapter on 64-127, where gcol also
                  # folds in the gating factor), then reshaped back
                  dsb = small_pool.tile([1, 2 * IB], F32, tag="dsb",
                                        name="dsb")
                  nc.vector.tensor_copy(out=dsb[:, :IB], in_=pden[:])
                  nc.vector.tensor_copy(out=dsb[:, IB:], in_=paden[:])
                  rsh = small_pool.tile([P, 2 * IB // P], F32, tag="rsh",
                                        name="rsh")
                  nc.sync.dma_start(
                      rsh[:], dsb.rearrange("x (p e) -> x p e", p=P))
                  rrecf = small_pool.tile([P, 2 * IB // P], F32, tag="rrecf",
                                          name="rrecf")
                  nc.vector.reciprocal(rrecf[:], rsh[:])
                  rrec = small_pool.tile([P, 2 * IB // P], BF16, tag="rrec",
                                         name="rrec")
                  with nc.allow_low_precision(
                          "softmax denominators are smooth sums of >=128 "
                          "probs; bf16 reciprocal costs ~0.4% on a tensor "
                          "checked at 2e-2"):
                      nc.vector.tensor_scalar_mul(rrec[:], rrecf[:],
                                                  gcol_sb[:])
                  rec = small_pool.tile([1, 2 * IB], BF16, tag="rec",
                                        name="rec")
                  nc.sync.dma_start(
                      rec.rearrange("x (p e) -> x p e", p=P), rrec[:])
                  pending[0] = (pyT, pyaT, rec, hh, ibb)

              def emit_normalize():
                  if pending[0] is None:
                      return
                  pyT, pyaT, rec, hh, ibb = pending[0]
                  pending[0] = None
                  pb = psST.tile([P, 2, IB], F32, tag="psT", name="pb")
                  nc.tensor.matmul(pb[:, 0, :], ones1[:], rec[:, :IB],
                                   start=True, stop=True)
                  nc.tensor.matmul(pb[:, 1, :], ones1[:], rec[:, IB:],
                                   start=True, stop=True)
                  b = bc_pool.tile([P, 2, IB], F32, tag="b", name="b")
                  nc.vector.tensor_copy(out=b[:], in_=pb[:])
                  ysl = yT_sb[:, hh, bass.ts(ibb, IB)]
                  nc.vector.tensor_mul(ysl, pyT[:], b[:, 0, :])
                  ya = bc_pool.tile([P, IB], BF16, tag="ya", name="ya")
                  nc.vector.tensor_mul(ya[:], pyaT[:], b[:, 1, :])
                  nc.vector.tensor_add(ysl, ysl, ya[:])

              for h in range(NH):
                  for ib in range(NIB):
                      unit_idx = h * NIB + ib
                      if unit_idx < 8:
                          load_wp_chunk(unit_idx)
                      isl = bass.ts(ib, IB)
                      nj = 4 * ib + 4
                      npair = nj // 2

                      # diagonal j-tiles first (their masks run early, and
                      # the group-closing stop matmul stays full-width on an
                      # off-diagonal tile); scores/exp/y/den for diagonal
                      # tile k only touch the valid i-columns [128k:512]
                      if ib == 0:
                          jt_order = [0, 1, 2, 3]
                      else:
                          jt_order = (list(range(4 * ib, 4 * ib + 4))
                                      + list(range(0, 4 * ib)))

                      def i_lo(jt, ib=ib, nj=nj):
                          k = jt - 4 * ib
                          if k < 0:
                              return 0
                          if ib == 0 and jt == 3:
                              return 0  # stop matmul must be full-width
                          return P * k

                      pyT = psYT.tile([P, IB], F32, tag="pyT", name="pyT")
                      pdena = psDen.tile([1, IB], F32, tag="pdena",
                                         name="pdena")

                      eTs = {}

                      def emit_scores_pair(pr, ib=ib, h=h, eTs=eTs,
                                           jt_order=jt_order, i_lo=i_lo):
                          # one 2-bank PSUM tile for two j-tiles; a single
                          # 1024-wide exp when both are full-width
                          psT = psST.tile([P, 2, IB], F32, tag="psT",
                                          name="psT")
                          los = []
                          for s in range(2):
                              jt = jt_order[2 * pr + s]
                              lo = i_lo(jt)
                              los.append(lo)
                              nc.tensor.matmul(
                                  psT[:, s, lo:],
                                  kT_sb[:, h, bass.ts(jt, P)],
                                  qT_sb[:, h, ib * IB + lo:(ib + 1) * IB],
                                  start=True, stop=True)
                          eT = exp_pool.tile([P, 2, IB], BF16, tag="eT",
                                             name="eT")
                          if los[0] == 0 and los[1] == 0:
                              nc.scalar.activation(eT[:], psT[:], EXP,
                                                   scale=SCALE)
                          else:
                              for s in range(2):
                                  nc.scalar.activation(
                                      eT[:, s, los[s]:], psT[:, s, los[s]:],
                                      EXP, scale=SCALE)
                          for s in range(2):
                              jt = jt_order[2 * pr + s]
                              k = jt - 4 * ib
                              if k < 0:
                                  continue
                              if ib == 0 and jt == 3:
                                  nc.vector.tensor_mul(
                                      eT[:, s, :], eT[:, s, :], mask3_sb[:])
                              else:
                                  lo = P * k
                                  nc.vector.tensor_mul(
                                      eT[:, s, lo:lo + P],
                                      eT[:, s, lo:lo + P], tri_sb[:])
                          eTs[pr] = eT

                      emit_scores_pair(0)
                      # adapter scores emitted after pair 0: exp(ea) still
                      # runs during the jt loop but no longer delays eT(0)
                      pasT = psST.tile([AT, IB], F32, tag="psT", name="pasT")
                      nc.tensor.matmul(pasT[:], akT_sb[:, h, :],
                                       qT_sb[:, h, isl],
                                       start=True, stop=True)
                      ea = exp_pool.tile([AT, IB], BF16, tag="ea", name="ea")
                      nc.scalar.activation(ea[:], pasT[:], EXP, scale=ASCALE)
                      if npair > 1:
                          emit_scores_pair(1)
                      first_jt = jt_order[0]
                      last_jt = jt_order[-1]
                      for pr in range(npair):
                          if pr + 2 < npair:
                              emit_scores_pair(pr + 2)
                          if pr == npair - 1:
                              emit_normalize()
                          eT = eTs[pr]
                          for s in range(2):
                              jt = jt_order[2 * pr + s]
                              lo = i_lo(jt)
                              nc.tensor.matmul(
                                  pyT[:, lo:], v_sb[:, jt, bass.ts(h, HD)],
                                  eT[:, s, lo:],
                                  start=(jt == first_jt),
                                  stop=(jt == last_jt))
                          jt0 = jt_order[2 * pr]
                          jt1 = jt_order[2 * pr + 1]
                          lo = max(i_lo(jt0), i_lo(jt1))
                          if lo > 0:
                              # partial-width pair: per-subtile accumulation
                              for s in range(2):
                                  jt = jt_order[2 * pr + s]
                                  lo_s = i_lo(jt)
                                  nc.tensor.matmul(
                                      pdena[0:1, lo_s:], ones128[:],
                                      eT[:, s, lo_s:],
                                      start=(jt == first_jt),
                                      stop=(jt == last_jt))
                          else:
                              # full-width pair: merge on DVE, single matmul
                              eTm = bc_pool.tile([P, IB], BF16, tag="eTm",
                                                 name="eTm")
                              nc.vector.tensor_add(eTm[:], eT[:, 0, :],
                                                   eT[:, 1, :])
                              nc.tensor.matmul(
                                  pdena[0:1, :], ones128[:], eTm[:],
                                  start=(jt0 == first_jt),
                                  stop=(jt1 == last_jt))
                          del eTs[pr]

                      paden = psST.tile([1, IB], F32, tag="psT",
                                        name="paden")
                      nc.tensor.matmul(paden[:], ones128[:AT, :],
                                       ea[:], start=True, stop=True)
                      pyaT = psYa.tile([P, IB], F32, tag="pyaT", name="pyaT")
                      nc.tensor.matmul(pyaT[:], av_sb[:, bass.ts(h, HD)],
                                       ea[:], start=True, stop=True)
                      emit_norm_stats(pyT, pyaT, pdena, paden, h, ib)

              emit_normalize()

          if dbg:
              nc.sync.dma_start(ydbg_d[:], yT_sb[:])

          # ============== output projection ====================
          with tc.tile_pool(name="outp", bufs=2) as out_pool, \
               tc.tile_pool(name="psD", bufs=4, space="PSUM") as psD:
              for tt in range(T // P):
                  osb = out_pool.tile([P, C], BF16, tag="osb", name="osb")
                  for ob in range(C // 512):
                      po = psD.tile([P, 512], F32, tag="po", name="po")
                      for hh in range(NH):
                          nc.tensor.matmul(
                              po[:],
                              yT_sb[:, hh, bass.ts(tt, P)],
                              wp_sb[:, hh, bass.ts(ob, 512)],
                              start=(hh == 0), stop=(hh == NH - 1))
                      if ob % 2 == 0:
                          nc.scalar.copy(osb[:, bass.ts(ob, 512)], po[:])
                      else:
                          nc.vector.tensor_copy(
                              out=osb[:, bass.ts(ob, 512)], in_=po[:])
                      if ob == 3:
                          nc.sync.dma_start(
                              out_d[bass.ts(tt, P), :2048], osb[:, :2048])
                  nc.sync.dma_start(out_d[bass.ts(tt, P), 2048:],
                                    osb[:, 2048:])

    nc.compile()
    return nc


_ROPE_CACHE = None


def _rope_cos_sin_T():
    global _ROPE_CACHE
    if _ROPE_CACHE is None:
        theta = 1.0 / (10000.0 ** (np.arange(0, HD, 2, dtype=np.float32) / HD))
        idx = np.outer(np.arange(T, dtype=np.float32), theta)  # [T, 64]
        full = np.concatenate([idx, idx], axis=1)              # [T, 128]
        _ROPE_CACHE = (np.ascontiguousarray(np.cos(full).T.astype(np.float32)),
                       np.ascontiguousarray(np.sin(full).T.astype(np.float32)))
    return _ROPE_CACHE


def _bf16(a):
    return np.ascontiguousarray(a.astype(ml_dtypes.bfloat16))


def kernel(x, w_attn, w_proj, adapter_wte, gating):
    x = np.asarray(x, np.float32)
    w_attn = np.asarray(w_attn, np.float32)
    w_proj = np.asarray(w_proj, np.float32)
    adapter_wte = np.asarray(adapter_wte, np.float32)
    gating = np.asarray(gating, np.float32)

    # x tiled [p, tb, ct, tcol]: xblk[p,tb,ct,i] = x[0, tb*TB+i, ct*P+p]
    xT = x[0].T                                               # [C, T]
    xblk = _bf16(xT.reshape(CT, P, NTB, TB).transpose(1, 2, 0, 3))
    # adapter wte tiled [p, ct, r]
    awteT = _bf16(adapter_wte.T.reshape(CT, P, AT).transpose(1, 0, 2))
    cosT, sinT = _rope_cos_sin_T()

    # rotate-half as a matmul: rot = R @ q (in [d, t] layout); pass R^T
    R = np.zeros((P, P), np.float32)
    for d in range(64):
        R[d, d + 64] = -1.0
        R[d + 64, d] = 1.0
    rotT = _bf16(R.T)

    # causal masks: tri for the [128j, 128i] triangle of each diagonal
    # j-tile; mask3 for the one full-width diagonal tile (ib=0, jt=3)
    pp = np.arange(P)[:, None]
    tri = _bf16((pp <= np.arange(P)[None, :]).astype(np.float32))
    mask3 = _bf16((pp + 3 * P <= np.arange(IB)[None, :]).astype(np.float32))

    # per-partition scale for the reshaped reciprocal: partitions 0-63 hold
    # self-attention denominators (x1), 64-127 adapter denominators (x g)
    gcol = np.ones((P, 1), np.float32)
    gcol[P // 2:] = float(gating[0])

    nc = _build_nc()

    def _pack_w(w):    # [ow, C] -> [p, ct, ow]
        return w.T.reshape(CT, P, -1).transpose(1, 0, 2)

    in_maps = []
    for m in range(NCORES):
        wq = w_attn[OW * m: OW * (m + 1)]
        wk = w_attn[C + OW * m: C + OW * (m + 1)]
        wv = w_attn[2 * C + OW * m: 2 * C + OW * (m + 1)]
        wqkT = _bf16(np.concatenate([_pack_w(wq), _pack_w(wk)], axis=2))
        wvT = _bf16(_pack_w(wv))
        # wpT[p, hh, o] = w_proj[o, m*OW + hh*P + p]
        wps = w_proj[:, OW * m: OW * (m + 1)].T               # [OW, C]
        wpT = _bf16(wps.reshape(NH, P, C).transpose(1, 0, 2))
        in_maps.append({
            "xblk": xblk, "wqkT": wqkT, "wvT": wvT, "wpT": wpT,
            "awteT": awteT, "cosT": cosT, "sinT": sinT, "rotT": rotT,
            "tri": tri, "mask3": mask3, "gcol": gcol,
        })

    trace = bool(int(os.environ.get("BASS_KERNEL_TRACE", "0")))
    res = run_bass_kernel_spmd(nc, in_maps, core_ids=list(range(NCORES)),
                               trace=trace)
    if trace:
        print("HW exec time:", res.exec_time_ns, "ns")
        print("trace:", res.instructions_and_trace[1]
              if res.instructions_and_trace else None)

    out = np.zeros((T, C), np.float64)
    for r in res.results:
        out += r["out"].astype(np.float64)
    return out.astype(np.float32)[None]


# revision 33
# speedup vs baseline: 1.0016x; 1.0016x over previous
"""Trainium2 Bass kernel for nn_CausalSelfAttention_8443905704568.

Causal self-attention with RoPE + 10-token adapter cross-attention,
B=1, T=2048, C=4096, H=32 heads of hd=128, fp32 I/O.

Strategy: tensor-parallel over heads across 8 NeuronCores (4 heads/core).
Each core computes qkv for its heads (w_attn rows sharded), runs
flash-style attention in transposed orientation (scores^T, so the
probs land partition=key which feeds the P^T @ V matmul directly),
the adapter cross-attention, and a partial output projection
(w_proj columns sharded). Host sums the 8 partial outputs.

v2 layout: everything bf16 (PE does 1 cyc/col at any moving-dim size,
half the HBM traffic of f32r; end-to-end error ~1e-3 vs the 2e-2 gate).
One fused pass over x computes q, k and v per t-block with all three
weight matrices SBUF-resident; q/k (roped) and v stay in SBUF for the
attention phase -- no DRAM staging round-trip at all.  All device
inputs are host-prepacked into [128, ...] partition-major tiles so
every DMA is a full-line contiguous copy.  w_proj streams in during
attention; the projection writes bf16 partials that the host sums.

Perf structure:
- startup: x block 0 chunks interleave with the first wqk chunks at the
  head of the DMA queue; the exp activation table is warmed by a dummy
  exp during that wait; the tiny adapter-column strip DMAs (2048 20B
  descriptors) are pushed behind the bulk weight transfers
- fused pass: per t-block q heads -> k heads -> v sub-tiles; tb0 runs
  ct-outer on q so matmuls start after the first weight chunk; adapter
  K rides tb0's k matmuls as 10 extra moving columns; 4 x-block
  buffers so the next block's DMA runs under the current compute
- attention: diagonal j-tiles are processed first and restricted to
  their valid i-columns (only a [128,128] triangle mask remains, on
  the vector engine); scores pairs are software-pipelined two ahead of
  their exp; the denominator rides one ones-matmul per pair on a
  DVE-merged eT sum; softmax normalization (reciprocal via a [1,1024]
  -> [128,8] DMA reshape, gating folded in per-partition) is deferred
  one unit so the PE never waits on it; w_proj streams in 0.5MB chunks
  between the first attention units
- projection: 4-deep PSUM rotation, bf16 partial out flushed per half
  row-tile

Everything is hardcoded for the shapes above; host-side prep only
slices/transposes/casts inputs (layout, not model compute).
"""

import math
import os

import ml_dtypes
import numpy as np

import concourse.bass as bass
import concourse.mybir as mybir
import concourse.tile as tile
from concourse import bacc
from concourse.bass_utils import run_bass_kernel_spmd

F32 = mybir.dt.float32
BF16 = mybir.dt.bfloat16

T = 2048
C = 4096
NHEAD_TOTAL = 32
NCORES = 8
NH = NHEAD_TOTAL // NCORES      # heads per core = 4
HD = C // NHEAD_TOTAL           # head dim = 128
P = 128
CT = C // P                     # contraction tiles = 32
OW = NH * HD                    # per-pass weight output dim = 512
TB = 256                        # x t-block
NTB = T // TB                   # 8
IB = 512                        # attention i-block
NIB = T // IB                   # 4
NJT = T // P                    # 16 j-tiles
AT = 10                         # adapter tokens
SCALE = 1.0 / math.sqrt(HD)
ASCALE = 1.0 / math.sqrt(C)
EXP = mybir.ActivationFunctionType.Exp
CQ = CT // 4                    # weight DMA chunk = 8 ct tiles


def _build_nc(dbg=False):
    nc = bacc.Bacc("TRN2", target_bir_lowering=False, debug=False,
                   num_devices=NCORES)

    xblk_d = nc.dram_tensor("xblk", [P, NTB, CT, TB], BF16,
                            kind="ExternalInput").ap()
    wqkT_d = nc.dram_tensor("wqkT", [P, CT, 2 * OW], BF16,
                            kind="ExternalInput").ap()
    wvT_d = nc.dram_tensor("wvT", [P, CT, OW], BF16,
                           kind="ExternalInput").ap()
    wpT_d = nc.dram_tensor("wpT", [P, NH, C], BF16,
                           kind="ExternalInput").ap()
    awteT_d = nc.dram_tensor("awteT", [P, CT, AT], BF16,
                             kind="ExternalInput").ap()
    cosT_d = nc.dram_tensor("cosT", [P, T], F32, kind="ExternalInput").ap()
    sinT_d = nc.dram_tensor("sinT", [P, T], F32, kind="ExternalInput").ap()
    rotT_d = nc.dram_tensor("rotT", [P, P], BF16, kind="ExternalInput").ap()
    tri_d = nc.dram_tensor("tri", [P, P], BF16, kind="ExternalInput").ap()
    mask3_d = nc.dram_tensor("mask3", [P, IB], BF16,
                             kind="ExternalInput").ap()
    gcol_d = nc.dram_tensor("gcol", [P, 1], F32, kind="ExternalInput").ap()

    out_d = nc.dram_tensor("out", [T, C], BF16, kind="ExternalOutput").ap()
    if dbg:
        qdbg_d = nc.dram_tensor("qdbg", [P, NH, T], BF16,
                                kind="ExternalOutput").ap()
        kdbg_d = nc.dram_tensor("kdbg", [P, NH, T], BF16,
                                kind="ExternalOutput").ap()
        vdbg_d = nc.dram_tensor("vdbg", [P, NJT, OW], BF16,
                                kind="ExternalOutput").ap()
        akdbg_d = nc.dram_tensor("akdbg", [P, NH, AT], BF16,
                                 kind="ExternalOutput").ap()
        avdbg_d = nc.dram_tensor("avdbg", [AT, OW], BF16,
                                 kind="ExternalOutput").ap()
        ydbg_d = nc.dram_tensor("ydbg", [P, NH, T], BF16,
                                kind="ExternalOutput").ap()

    with tile.TileContext(nc) as tc:
      # persistent staging: q/k (roped) + v + y, all bf16, SBUF-resident
      with tc.tile_pool(name="stage", bufs=1) as stage, \
           tc.tile_pool(name="const", bufs=1) as cpool:
        qT_sb = stage.tile([P, NH, T], BF16, name="qT_sb")
        kT_sb = stage.tile([P, NH, T], BF16, name="kT_sb")
        v_sb = stage.tile([P, NJT, OW], BF16, name="v_sb")
        yT_sb = stage.tile([P, NH, T], BF16, name="yT_sb")

        awteT_sb = cpool.tile([P, CT, AT], BF16, name="awteT_sb")
        gcol_sb = cpool.tile([P, 1], F32, name="gcol_sb")
        ones128f = cpool.tile([P, 1], F32, name="ones128f")
        ones1f = cpool.tile([1, P], F32, name="ones1f")
        ones128 = cpool.tile([P, 1], BF16, name="ones128")
        ones1 = cpool.tile([1, P], BF16, name="ones1")
        akT_sb = cpool.tile([P, NH, AT], BF16, name="akT_sb")
        av_sb = cpool.tile([AT, OW], BF16, name="av_sb")
        tri_sb = cpool.tile([P, P], BF16, name="tri_sb")
        mask3_sb = cpool.tile([P, IB], BF16, name="mask3_sb")
        warm_sb = cpool.tile([1, 8], BF16, name="warm_sb")

        # ===== fused qkv pass (x read once, weights all resident) ========
        with tc.tile_pool(name="arot", bufs=1) as arot_pool, \
             tc.tile_pool(name="wpool", bufs=1) as w_pool, \
             tc.tile_pool(name="cs", bufs=2) as cs_pool, \
             tc.tile_pool(name="xa", bufs=4) as xa_pool, \
             tc.tile_pool(name="ropeA", bufs=2) as rope_pool, \
             tc.tile_pool(name="psA", bufs=4, space="PSUM") as psA, \
             tc.tile_pool(name="psArot", bufs=2, space="PSUM") as psArot, \
             tc.tile_pool(name="psAv", bufs=2, space="PSUM") as psAv:

            wqk_sb = w_pool.tile([P, CT, 2 * OW], BF16, name="wqk_sb")
            wv_sb = w_pool.tile([P, CT, OW], BF16, name="wv_sb")
            rotT_sb = arot_pool.tile([P, P], BF16, name="rotT_sb")

            def load_cs(tsl):
                cos_t = cs_pool.tile([P, TB], F32, tag="cos", name="cos_t")
                sin_t = cs_pool.tile([P, TB], F32, tag="sin", name="sin_t")
                nc.sync.dma_start(cos_t[:], cosT_d[:, tsl])
                nc.sync.dma_start(sin_t[:], sinT_d[:, tsl])
                return cos_t, sin_t

            def load_x_block(tb, chunks=1):
                # tb0 tiles carry the 10 adapter-wte columns appended to the
                # x columns, so adapter K rides the k matmuls as extra
                # moving columns of the same accumulation group
                wide = (tb == 0)
                ncol = TB + AT if wide else TB
                xh = []
                for half in range(2):
                    base = half * (CT // 2)
                    xa = xa_pool.tile([P, CT // 2, ncol], BF16, tag="xa",
                                      name="xa")
                    cn = (CT // 2) // chunks
                    for c in range(chunks):
                        nc.sync.dma_start(
                            xa[:, c * cn:(c + 1) * cn, :TB],
                            xblk_d[:, tb, base + c * cn:base + (c + 1) * cn,
                                   :])
                    xh.append(xa)
                return xh

            def load_adapter_strips(xh):
                # 10-wide strips -> tiny DMA descriptors; issued after the
                # weight chunks so they never sit ahead of them in the queue
                for half in range(2):
                    base = half * (CT // 2)
                    nc.sync.dma_start(xh[half][:, :, TB:],
                                      awteT_d[:, base:base + CT // 2, :])

            # DMA issue order: x block 0 + first wqk chunks fine-grained
            # (what the first matmuls wait on), then everything else.
            xh0 = []
            for half in range(2):
                xa = xa_pool.tile([P, CT // 2, TB + AT], BF16, tag="xa",
                                  name="xa")
                xh0.append(xa)
            for c in range(4):
                nc.sync.dma_start(
                    xh0[0][:, c * 4:(c + 1) * 4, :TB],
                    xblk_d[:, 0, c * 4:(c + 1) * 4, :])
                nc.sync.dma_start(wqk_sb[:, bass.ts(c, 2), :],
                                  wqkT_d[:, bass.ts(c, 2), :])
                nc.sync.dma_start(
                    xh0[1][:, c * 4:(c + 1) * 4, :TB],
                    xblk_d[:, 0, 16 + c * 4:16 + (c + 1) * 4, :])
            cs0 = load_cs(bass.ts(0, TB))
            for c4 in range(1, 4):
                nc.sync.dma_start(wqk_sb[:, bass.ts(c4, CQ), :],
                                  wqkT_d[:, bass.ts(c4, CQ), :])
            for c4 in range(4):
                nc.sync.dma_start(wv_sb[:, bass.ts(c4, CQ), :],
                                  wvT_d[:, bass.ts(c4, CQ), :])
            nc.sync.dma_start(rotT_sb[:], rotT_d[:])
            load_adapter_strips(xh0)
            nc.sync.dma_start(awteT_sb[:], awteT_d[:])
            nc.sync.dma_start(tri_sb[:], tri_d[:])
            nc.sync.dma_start(mask3_sb[:], mask3_d[:])
            nc.sync.dma_start(gcol_sb[:], gcol_d[:])

            nc.vector.memset(ones128f[:], 1.0)
            nc.vector.memset(ones1f[:], 1.0)
            nc.vector.tensor_copy(out=ones128[:], in_=ones128f[:])
            nc.vector.tensor_copy(out=ones1[:], in_=ones1f[:])
            # warm the exp activation-table set during the initial DMA wait
            nc.scalar.activation(warm_sb[:], ones1f[:, :8], EXP)

            def rope_tail(pqk, dst, oh, tsl, wide, cos_t, sin_t):
                # dst[:, oh, tsl] = pqk[:, :TB]*cos + rot(pqk)*sin
                if wide:
                    nc.scalar.copy(akT_sb[:, oh, :], pqk[:, TB:])
                raw = rope_pool.tile([P, TB], BF16, tag="raw", name="raw")
                nc.scalar.copy(raw[:], pqk[:, :TB])
                prot = psArot.tile([P, TB], F32, tag="prot", name="prot")
                nc.tensor.matmul(prot[:], rotT_sb[:], raw[:],
                                 start=True, stop=True)
                t1 = rope_pool.tile([P, TB], BF16, tag="t1", name="t1")
                nc.vector.tensor_mul(t1[:], prot[:], sin_t[:])
                dsl = dst[:, oh, tsl]
                nc.vector.tensor_mul(dsl, pqk[:, :TB], cos_t[:])
                nc.vector.tensor_add(dsl, dsl, t1[:])

            for tb in range(NTB):
                tsl = bass.ts(tb, TB)
                xh = xh0 if tb == 0 else load_x_block(tb)
                wide = (tb == 0)
                cos_t, sin_t = cs0 if tb == 0 else load_cs(tsl)

                # ---- q heads ----
                if tb == 0:
                    # ct-outer: matmuls start as soon as the first weight
                    # chunk + x half arrive
                    pqks = [psA.tile([P, TB], F32, tag="pqk",
                                     name=f"pqk{oh}") for oh in range(NH)]
                    for ct in range(CT):
                        for oh in range(NH):
                            nc.tensor.matmul(
                                pqks[oh][:],
                                wqk_sb[:, ct, oh * HD:(oh + 1) * HD],
                                xh[ct // (CT // 2)][:, ct % (CT // 2), :TB],
                                start=(ct == 0), stop=(ct == CT - 1))
                    for oh in range(NH):
                        rope_tail(pqks[oh], qT_sb, oh, tsl, False, cos_t, sin_t)
                else:
                    for oh in range(NH):
                        pqk = psA.tile([P, TB], F32, tag="pqk", name="pqk")
                        for ct in range(CT):
                            nc.tensor.matmul(
                                pqk[:],
                                wqk_sb[:, ct, oh * HD:(oh + 1) * HD],
                                xh[ct // (CT // 2)][:, ct % (CT // 2), :TB],
                                start=(ct == 0), stop=(ct == CT - 1))
                        rope_tail(pqk, qT_sb, oh, tsl, False, cos_t, sin_t)

                # ---- k heads (adapter K rides tb0 as 10 extra columns) ----
                ncol = TB + AT if wide else TB
                for oh in range(NH):
                    pqk = psA.tile([P, ncol], F32, tag="pqk", name="pqkk")
                    for ct in range(CT):
                        nc.tensor.matmul(
                            pqk[:],
                            wqk_sb[:, ct, OW + oh * HD:OW + (oh + 1) * HD],
                            xh[ct // (CT // 2)][:, ct % (CT // 2), :],
                            start=(ct == 0), stop=(ct == CT - 1))
                    rope_tail(pqk, kT_sb, oh, tsl, wide, cos_t, sin_t)

                # ---- v sub-tiles ----
                for sub in range(TB // P):
                    tt = tb * (TB // P) + sub
                    pv = psAv.tile([P, OW], F32, tag="pv", name="pv")
                    for ct in range(CT):
                        nc.tensor.matmul(
                            pv[:],
                            xh[ct // (CT // 2)][:, ct % (CT // 2),
                                                sub * P:(sub + 1) * P],
                            wv_sb[:, ct, :],
                            start=(ct == 0), stop=(ct == CT - 1))
                    nc.scalar.copy(v_sb[:, tt, :], pv[:])

            # adapter-v
            pav = psAv.tile([AT, OW], F32, tag="pv", name="pav")
            for ct in range(CT):
                nc.tensor.matmul(pav[:], awteT_sb[:, ct, :], wv_sb[:, ct, :],
                                 start=(ct == 0), stop=(ct == CT - 1))
            nc.scalar.copy(av_sb[:], pav[:])

        if dbg:
            nc.sync.dma_start(qdbg_d[:], qT_sb[:])
            nc.sync.dma_start(kdbg_d[:], kT_sb[:])
            nc.sync.dma_start(vdbg_d[:], v_sb[:])
            nc.sync.dma_start(akdbg_d[:], akT_sb[:])
            nc.sync.dma_start(avdbg_d[:], av_sb[:])

        # ================= attention per head (all SBUF-resident) =========
        with tc.tile_pool(name="wp2", bufs=1) as wp2_pool:
          wp_sb = wp2_pool.tile([P, NH, C], BF16, name="wp_sb")

          def load_wp_chunk(i):
              # 0.5MB chunks interleaved between attention units so they
              # never delay the small normalization DMAs behind them
              hh, half = divmod(i, 2)
              csl = bass.ts(half, C // 2)
              nc.sync.dma_start(wp_sb[:, hh, csl], wpT_d[:, hh, csl])

          with tc.tile_pool(name="expp", bufs=6) as exp_pool, \
               tc.tile_pool(name="small", bufs=2) as small_pool, \
               tc.tile_pool(name="bcast", bufs=2) as bc_pool, \
               tc.tile_pool(name="psST", bufs=2, space="PSUM") as psST, \
               tc.tile_pool(name="psYT", bufs=2, space="PSUM") as psYT, \
               tc.tile_pool(name="psDen", bufs=1, space="PSUM") as psDen, \
               tc.tile_pool(name="psYa", bufs=1, space="PSUM") as psYa:

              pending = [None]  # deferred normalization of the previous block

              def emit_norm_stats(pyT, ya_raw, pden, paden, hh, ibb):
                  # off the PE critical path: both denominators go into one
                  # [1, 2*512] row, reshaped to [128, 8] by DMA so the
                  # reciprocal runs on all lanes (self-denominators land on
                  # partitions 0-63, adapter on 64-127, where gcol also
                  # folds in the gating factor), then reshaped back
                  dsb = small_pool.tile([1, 2 * IB], F32, tag="dsb",
                                        name="dsb")
                  nc.vector.tensor_copy(out=dsb[:, :IB], in_=pden[:])
                  nc.vector.tensor_copy(out=dsb[:, IB:], in_=paden[:])
                  rsh = small_pool.tile([P, 2 * IB // P], F32, tag="rsh",
                                        name="rsh")
                  nc.sync.dma_start(
                      rsh[:], dsb.rearrange("x (p e) -> x p e", p=P))
                  rrecf = small_pool.tile([P, 2 * IB // P], F32, tag="rrecf",
                                          name="rrecf")
                  nc.vector.reciprocal(rrecf[:], rsh[:])
                  rrec = small_pool.tile([P, 2 * IB // P], BF16, tag="rrec",
                                         name="rrec")
                  with nc.allow_low_precision(
                          "softmax denominators are smooth sums of >=128 "
                          "probs; bf16 reciprocal costs ~0.4% on a tensor "
                          "checked at 2e-2"):
                      nc.vector.tensor_scalar_mul(rrec[:], rrecf[:],
                                                  gcol_sb[:])
                  rec = small_pool.tile([1, 2 * IB], BF16, tag="rec",
                                        name="rec")
                  nc.sync.dma_start(
                      rec.rearrange("x (p e) -> x p e", p=P), rrec[:])
                  pending[0] = (pyT, ya_raw, rec, hh, ibb)

              def emit_normalize():
                  if pending[0] is None:
                      return
                  pyT, ya_raw, rec, hh, ibb = pending[0]
                  pending[0] = None
                  pb = psST.tile([P, 2, IB], F32, tag="psT", name="pb")
                  nc.tensor.matmul(pb[:, 0, :], ones1[:], rec[:, :IB],
                                   start=True, stop=True)
                  nc.tensor.matmul(pb[:, 1, :], ones1[:], rec[:, IB:],
                                   start=True, stop=True)
                  b = bc_pool.tile([P, 2, IB], F32, tag="b", name="b")
                  nc.vector.tensor_copy(out=b[:], in_=pb[:])
                  ysl = yT_sb[:, hh, bass.ts(ibb, IB)]
                  nc.vector.tensor_mul(ysl, pyT[:], b[:, 0, :])
                  ya = bc_pool.tile([P, IB], BF16, tag="ya", name="ya")
                  nc.gpsimd.tensor_mul(ya[:], ya_raw[:], b[:, 1, :])
                  nc.gpsimd.tensor_add(ysl, ysl, ya[:])

              for h in range(NH):
                  for ib in range(NIB):
                      unit_idx = h * NIB + ib
                      if unit_idx < 8:
                          load_wp_chunk(unit_idx)
                      isl = bass.ts(ib, IB)
                      nj = 4 * ib + 4
                      npair = nj // 2

                      # diagonal j-tiles first (their masks run early, and
                      # the group-closing stop matmul stays full-width on an
                      # off-diagonal tile); scores/exp/y/den for diagonal
                      # tile k only touch the valid i-columns [128k:512]
                      if ib == 0:
                          jt_order = [0, 1, 2, 3]
                      else:
                          jt_order = (list(range(4 * ib, 4 * ib + 4))
                                      + list(range(0, 4 * ib)))

                      def i_lo(jt, ib=ib, nj=nj):
                          k = jt - 4 * ib
                          if k < 0:
                              return 0
                          if ib == 0 and jt == 3:
                              return 0  # stop matmul must be full-width
                          return P * k

                      pyT = psYT.tile([P, IB], F32, tag="pyT", name="pyT")
                      pdena = psDen.tile([1, IB], F32, tag="pdena",
                                         name="pdena")

                      eTs = {}

                      def emit_scores_pair(pr, ib=ib, h=h, eTs=eTs,
                                           jt_order=jt_order, i_lo=i_lo):
                          # one 2-bank PSUM tile for two j-tiles; a single
                          # 1024-wide exp when both are full-width
                          psT = psST.tile([P, 2, IB], F32, tag="psT",
                                          name="psT")
                          los = []
                          for s in range(2):
                              jt = jt_order[2 * pr + s]
                              lo = i_lo(jt)
                              los.append(lo)
                              nc.tensor.matmul(
                                  psT[:, s, lo:],
                                  kT_sb[:, h, bass.ts(jt, P)],
                                  qT_sb[:, h, ib * IB + lo:(ib + 1) * IB],
                                  start=True, stop=True)
                          eT = exp_pool.tile([P, 2, IB], BF16, tag="eT",
                                             name="eT")
                          if los[0] == 0 and los[1] == 0:
                              nc.scalar.activation(eT[:], psT[:], EXP,
                                                   scale=SCALE)
                          else:
                              for s in range(2):
                                  nc.scalar.activation(
                                      eT[:, s, los[s]:], psT[:, s, los[s]:],
                                      EXP, scale=SCALE)
                          for s in range(2):
                              jt = jt_order[2 * pr + s]
                              k = jt - 4 * ib
                              if k < 0:
                                  continue
                              if ib == 0 and jt == 3:
                                  nc.vector.tensor_mul(
                                      eT[:, s, :], eT[:, s, :], mask3_sb[:])
                              else:
                                  lo = P * k
                                  nc.vector.tensor_mul(
                                      eT[:, s, lo:lo + P],
                                      eT[:, s, lo:lo + P], tri_sb[:])
                          eTs[pr] = eT

                      emit_scores_pair(0)
                      # adapter scores emitted after pair 0: exp(ea) still
                      # runs during the jt loop but no longer delays eT(0)
                      pasT = psST.tile([AT, IB], F32, tag="psT", name="pasT")
                      nc.tensor.matmul(pasT[:], akT_sb[:, h, :],
                                       qT_sb[:, h, isl],
                                       start=True, stop=True)
                      ea = exp_pool.tile([AT, IB], BF16, tag="ea", name="ea")
                      nc.scalar.activation(ea[:], pasT[:], EXP, scale=ASCALE)
                      if npair > 1:
                          emit_scores_pair(1)
                      first_jt = jt_order[0]
                      last_jt = jt_order[-1]
                      for pr in range(npair):
                          if pr + 2 < npair:
                              emit_scores_pair(pr + 2)
                          if pr == npair - 1:
                              emit_normalize()
                          eT = eTs[pr]
                          for s in range(2):
                              jt = jt_order[2 * pr + s]
                              lo = i_lo(jt)
                              nc.tensor.matmul(
                                  pyT[:, lo:], v_sb[:, jt, bass.ts(h, HD)],
                                  eT[:, s, lo:],
                                  start=(jt == first_jt),
                                  stop=(jt == last_jt))
                          jt0 = jt_order[2 * pr]
                          jt1 = jt_order[2 * pr + 1]
                          lo = max(i_lo(jt0), i_lo(jt1))
                          if lo > 0:
                              # partial-width pair: per-subtile accumulation
                              for s in range(2):
                                  jt = jt_order[2 * pr + s]
                                  lo_s = i_lo(jt)
                                  nc.tensor.matmul(
                                      pdena[0:1, lo_s:], ones128[:],
                                      eT[:, s, lo_s:],
                                      start=(jt == first_jt),
                                      stop=(jt == last_jt))
                          else:
                              # full-width pair: merge on DVE, single matmul
                              eTm = bc_pool.tile([P, IB], BF16, tag="eTm",
                                                 name="eTm")
                              nc.vector.tensor_add(eTm[:], eT[:, 0, :],
                                                   eT[:, 1, :])
                              nc.tensor.matmul(
                                  pdena[0:1, :], ones128[:], eTm[:],
                                  start=(jt0 == first_jt),
                                  stop=(jt1 == last_jt))
                          del eTs[pr]

                      paden = psST.tile([1, IB], F32, tag="psT",
                                        name="paden")
                      nc.tensor.matmul(paden[:], ones128[:AT, :],
                                       ea[:], start=True, stop=True)
                      pyaT = psYa.tile([P, IB], F32, tag="pyaT", name="pyaT")
                      nc.tensor.matmul(pyaT[:], av_sb[:, bass.ts(h, HD)],
                                       ea[:], start=True, stop=True)
                      # copy out of PSUM immediately so the single psYa bank
                      # frees before the deferred normalization runs
                      ya_raw = bc_pool.tile([P, IB], F32, tag="ya_raw",
                                            name="ya_raw")
                      nc.vector.tensor_copy(out=ya_raw[:], in_=pyaT[:])
                      emit_norm_stats(pyT, ya_raw, pdena, paden, h, ib)

              emit_normalize()

          if dbg:
              nc.sync.dma_start(ydbg_d[:], yT_sb[:])

          # ============== output projection ====================
          with tc.tile_pool(name="outp", bufs=2) as out_pool, \
               tc.tile_pool(name="psD", bufs=4, space="PSUM") as psD:
              for tt in range(T // P):
                  osb = out_pool.tile([P, C], BF16, tag="osb", name="osb")
                  for ob in range(C // 512):
                      po = psD.tile([P, 512], F32, tag="po", name="po")
                      for hh in range(NH):
                          nc.tensor.matmul(
                              po[:],
                              yT_sb[:, hh, bass.ts(tt, P)],
                              wp_sb[:, hh, bass.ts(ob, 512)],
                              start=(hh == 0), stop=(hh == NH - 1))
                      if ob % 2 == 0:
                          nc.scalar.copy(osb[:, bass.ts(ob, 512)], po[:])
                      else:
                          nc.vector.tensor_copy(
                              out=osb[:, bass.ts(ob, 512)], in_=po[:])
                      if ob == 3:
                          nc.sync.dma_start(
                              out_d[bass.ts(tt, P), :2048], osb[:, :2048])
                  nc.sync.dma_start(out_d[bass.ts(tt, P), 2048:],
                                    osb[:, 2048:])

    nc.compile()
    return nc


_ROPE_CACHE = None


def _rope_cos_sin_T():
    global _ROPE_CACHE
    if _ROPE_CACHE is None:
        theta = 1.0 / (10000.0 ** (np.arange(0, HD, 2, dtype=np.float32) / HD))
        idx = np.outer(np.arange(T, dtype=np.float32), theta)  # [T, 64]
        full = np.concatenate([idx, idx], axis=1)              # [T, 128]
        _ROPE_CACHE = (np.ascontiguousarray(np.cos(full).T.astype(np.float32)),
                       np.ascontiguousarray(np.sin(full).T.astype(np.float32)))
    return _ROPE_CACHE


def _bf16(a):
    return np.ascontiguousarray(a.astype(ml_dtypes.bfloat16))


def kernel(x, w_attn, w_proj, adapter_wte, gating):
    x = np.asarray(x, np.float32)
    w_attn = np.asarray(w_attn, np.float32)
    w_proj = np.asarray(w_proj, np.float32)
    adapter_wte = np.asarray(adapter_wte, np.float32)
    gating = np.asarray(gating, np.float32)

    # x tiled [p, tb, ct, tcol]: xblk[p,tb,ct,i] = x[0, tb*TB+i, ct*P+p]
    xT = x[0].T                                               # [C, T]
    xblk = _bf16(xT.reshape(CT, P, NTB, TB).transpose(1, 2, 0, 3))
    # adapter wte tiled [p, ct, r]
    awteT = _bf16(adapter_wte.T.reshape(CT, P, AT).transpose(1, 0, 2))
    cosT, sinT = _rope_cos_sin_T()

    # rotate-half as a matmul: rot = R @ q (in [d, t] layout); pass R^T
    R = np.zeros((P, P), np.float32)
    for d in range(64):
        R[d, d + 64] = -1.0
        R[d + 64, d] = 1.0
    rotT = _bf16(R.T)

    # causal masks: tri for the [128j, 128i] triangle of each diagonal
    # j-tile; mask3 for the one full-width diagonal tile (ib=0, jt=3)
    pp = np.arange(P)[:, None]
    tri = _bf16((pp <= np.arange(P)[None, :]).astype(np.float32))
    mask3 = _bf16((pp + 3 * P <= np.arange(IB)[None, :]).astype(np.float32))

    # per-partition scale for the reshaped reciprocal: partitions 0-63 hold
    # self-attention denominators (x1), 64-127 adapter denominators (x g)
    gcol = np.ones((P, 1), np.float32)
    gcol[P // 2:] = float(gating[0])

    nc = _build_nc()

    def _pack_w(w):    # [ow, C] -> [p, ct, ow]
        return w.T.reshape(CT, P, -1).transpose(1, 0, 2)

    in_maps = []
    for m in range(NCORES):
        wq = w_attn[OW * m: OW * (m + 1)]
        wk = w_attn[C + OW * m: C + OW * (m + 1)]
        wv = w_attn[2 * C + OW * m: 2 * C + OW * (m + 1)]
        wqkT = _bf16(np.concatenate([_pack_w(wq), _pack_w(wk)], axis=2))
        wvT = _bf16(_pack_w(wv))
        # wpT[p, hh, o] = w_proj[o, m*OW + hh*P + p]
        wps = w_proj[:, OW * m: OW * (m + 1)].T               # [OW, C]
        wpT = _bf16(wps.reshape(NH, P, C).transpose(1, 0, 2))
        in_maps.append({
            "xblk": xblk, "wqkT": wqkT, "wvT": wvT, "wpT": wpT,
            "awteT": awteT, "cosT": cosT, "sinT": sinT, "rotT": rotT,
            "tri": tri, "mask3": mask3, "gcol": gcol,
        })

    trace = bool(int(os.environ.get("BASS_KERNEL_TRACE", "0")))
    res = run_bass_kernel_spmd(nc, in_maps, core_ids=list(range(NCORES)),
                               trace=trace)
    if trace:
        print("HW exec time:", res.exec_time_ns, "ns")
        print("trace:", res.instructions_and_trace[1]
              if res.instructions_and_trace else None)

    out = np.zeros((T, C), np.float64)
    for r in res.results:
        out += r["out"].astype(np.float64)
    return out.astype(np.float32)[None]
